# revision 36
# baseline (speedup 1.0000x reference)
"""Causal multi-head attention (B=2, L=2048, D=2048, NH=16, HD=128) on 8
Trainium2 NeuronCores.

Sharding: core c = b*4 + g handles batch b and head-group g (4 heads).
Each core computes q/k/v projections for its 512 features, causal
attention for its 4 heads, and the partial o-projection
attn_out @ Wo[:, g_cols].T -> [L, D] (fp16).  The host sums the 4
per-batch partials and adds bo.

v2 structure:
  phase 1: q,k AND v projections for the whole sequence from the same
           streamed x chunks (x read from HBM once); wq/wk/wv resident.
           qT/kT/v stored fp16.
  phase 2: flash-style causal attention in [k, q] layout fused with the
           partial o-projection.  Softmax row-sums are accumulated on
           the (otherwise idle) DVE engine across j-tiles, then reduced
           across partitions with ONE ones-matmul per (q-block, head)
           instead of one per j-tile.  pt/att/wo are fp16 so every
           matmul's moving operand runs at 1 cycle/row even when only
           128 columns survive the causal trim.

Matmul dtypes: moving operands are fp16 or f32r-with-moving>=256 (both
full rate).  PSUM accumulation is fp32 throughout.
"""

import sys

for _p in ("/opt/trn_rl_repo",):
    if _p not in sys.path:
        sys.path.insert(0, _p)

import numpy as np
from contextlib import ExitStack

import concourse.bass as bass  # noqa: F401
import concourse.tile as tile
from concourse import bacc, mybir
from concourse import bass_utils

P = 128
B, L, D = 2, 2048, 2048
NH, HD = 16, 128
SCALE = HD ** -0.5
G = 8 // B            # head-groups per batch = 4
H = 4                 # heads per core
F = H * HD            # 512 features per core
TB = 512              # token block (q-block)
NTB = L // TB         # 4
KT = D // P           # 16 contraction tiles for projections

f32r = mybir.dt.float32r
f32 = mybir.dt.float32
fp16 = mybir.dt.float16

_CACHE = {}


def _build(reps=1):
    key = ("nc", reps)
    if key in _CACHE:
        return _CACHE[key]

    nc = bacc.Bacc("TRN2", target_bir_lowering=False, debug=False, num_devices=8)

    xT = nc.dram_tensor("xT", [D, L], fp16, kind="ExternalInput").ap()
    wq = nc.dram_tensor("wq", [P, H, KT, HD], fp16, kind="ExternalInput").ap()
    wk = nc.dram_tensor("wk", [P, H, KT, HD], fp16, kind="ExternalInput").ap()
    wv = nc.dram_tensor("wv", [P, KT, F], fp16, kind="ExternalInput").ap()
    wo = nc.dram_tensor("wo", [P, H, D], fp16, kind="ExternalInput").ap()
    bqv = nc.dram_tensor("bqv", [F], f32r, kind="ExternalInput").ap()
    bkv = nc.dram_tensor("bkv", [F], f32r, kind="ExternalInput").ap()
    bvv = nc.dram_tensor("bvv", [F], fp16, kind="ExternalInput").ap()
    o = nc.dram_tensor("o", [L, D], fp16, kind="ExternalOutput").ap()

    xT3 = xT.rearrange("(kt p) t -> p kt t", p=P)

    with tile.TileContext(nc) as tc:
        with ExitStack() as ctx:
            ctx.enter_context(nc.allow_low_precision(reason="fp16 attention"))
            consts = ctx.enter_context(tc.tile_pool(name="consts", bufs=1))
            resid = ctx.enter_context(tc.tile_pool(name="resid", bufs=1))

            # ---- constants ----
            # triangular additive mask [P, P] in [k, q] orientation:
            # keep (0.0) where k_local <= q_local, else -1e30
            tri = consts.tile([P, P], f32, name="tri")
            nc.gpsimd.memset(tri[:], 0.0)
            nc.gpsimd.affine_select(
                out=tri[:],
                in_=tri[:],
                compare_op=mybir.AluOpType.is_ge,
                fill=-1e30,
                base=0,
                pattern=[[1, P]],
                channel_multiplier=-1,
            )

            # consts come from memset / the GPSIMD SWDGE queue: the SP and
            # ACT HWDGE queues are needed for the startup x/weight streams.
            # NeuronCC requires matmul operand dtypes to be both-32-bit or
            # both-non-32-bit: everything 16-bit is fp16 here.
            ones_col = consts.tile([P, 1], fp16)
            nc.gpsimd.memset(ones_col[:], 1.0)
            ones_row = consts.tile([1, P], fp16)
            nc.gpsimd.memset(ones_row[:], 1.0)
            bv_row = consts.tile([1, F], fp16)
            nc.gpsimd.dma_start(bv_row[:], bvv[None, :])
            # per-partition bias tiles for the q/k copies: [P, H]
            bq_pp = consts.tile([P, H], f32, name="bq_pp")
            nc.gpsimd.dma_start(bq_pp[:], bqv.rearrange("(h p) -> p h", p=P).bitcast(f32))
            bk_pp = consts.tile([P, H], f32, name="bk_pp")
            nc.gpsimd.dma_start(bk_pp[:], bkv.rearrange("(h p) -> p h", p=P).bitcast(f32))

            # ---- persistent activations (fp16) ----
            qT_sb = resid.tile([P, H, L], fp16, name="qT_sb")     # 2 MiB
            kT_sb = resid.tile([P, H, L], fp16, name="kT_sb")     # 2 MiB
            v_sb = resid.tile([P, L // P, F], fp16, name="v_sb")  # 2 MiB

            rep_ctx = ExitStack()
            if reps > 1:
                # timing mode: repeat the whole body in a hardware loop
                rep_ctx.enter_context(tc.For_i(0, reps, 1))

            # ============ phase 1: q, k, v projections ============
            # the SBUF pools stay open through phase 2: the last chunk's
            # v-quarters are deferred into the attention pipeline (they are
            # first needed by tb3) to fill the ACT-paced tb0 stretch
            p1sb = ExitStack()
            wres_pool = p1sb.enter_context(tc.tile_pool(name="wres", bufs=1))
            xpool = p1sb.enter_context(tc.tile_pool(name="xpool", bufs=3))
            with ExitStack() as p1:
                psA = p1.enter_context(tc.tile_pool(name="psA", bufs=4, space="PSUM"))
                psV = p1.enter_context(tc.tile_pool(name="psV", bufs=2, space="PSUM"))

                wq_res = wres_pool.tile([P, H, KT, HD], fp16, name="wq_res")
                wk_res = wres_pool.tile([P, H, KT, HD], fp16, name="wk_res")
                wv_res = wres_pool.tile([P, KT, F], fp16, name="wv_res")
                # split the weight stream across the two HWDGE queues in
                # consumption order: SP carries wq-h0 then the x pieces that
                # pace the first PSUM groups (plus the late heads); ACT
                # carries the early k/q heads and wv in parallel
                nc.sync.dma_start(wq_res[:, 0, :2], wq[:, 0, :2])

                for c in range(NTB):
                    lo = c * TB
                    xt = xpool.tile([P, KT, TB], fp16, tag="xT")
                    if c == 0:
                        # split the first chunk per-kt so the first PSUM
                        # group can start as soon as piece 0 lands; early
                        # weights stream on the ACT queue in parallel
                        nc.scalar.dma_start(wk_res[:, 0], wk[:, 0])
                        nc.scalar.dma_start(wq_res[:, 1], wq[:, 1])
                        nc.scalar.dma_start(wk_res[:, 1], wk[:, 1])
                        for kt in range(KT):
                            nc.sync.dma_start(xt[:, kt], xT3[:, kt, lo : lo + TB])
                            if kt == 1:
                                nc.sync.dma_start(wq_res[:, 0, 2:], wq[:, 0, 2:])
                        for vq in range(4):
                            nc.scalar.dma_start(
                                wv_res[:, 4 * vq : 4 * vq + 4],
                                wv[:, 4 * vq : 4 * vq + 4],
                            )
                        for hh in range(2, H):
                            nc.sync.dma_start(wq_res[:, hh], wq[:, hh])
                            nc.sync.dma_start(wk_res[:, hh], wk[:, hh])
                    else:
                        nc.sync.dma_start(xt[:], xT3[:, :, lo : lo + TB])
                    for h in range(H):
                        for wres, bias_pp, dst in (
                            (wq_res, bq_pp, qT_sb),
                            (wk_res, bk_pp, kT_sb),
                        ):
                            ps = psA.tile([P, TB], f32, tag="psA")
                            for kt in range(KT):
                                nc.tensor.matmul(
                                    ps[:],
                                    wres[:, h, kt],
                                    xt[:, kt],
                                    start=(kt == 0),
                                    stop=(kt == KT - 1),
                                )
                            nc.scalar.activation(
                                dst[:, h, lo : lo + TB],
                                ps[:],
                                mybir.ActivationFunctionType.Identity,
                                bias=bias_pp[:, h : h + 1],
                                scale=1.0,
                            )
                    if c == NTB - 1:
                        last_xt = xt
                        continue
                    for s in range(TB // P):
                        # v quarter: x slice stationary, wv moving
                        ps = psV.tile([P, F], f32, tag="psV")
                        for kt in range(KT):
                            nc.tensor.matmul(
                                ps[:],
                                xt[:, kt, s * P : (s + 1) * P],
                                wv_res[:, kt],
                                start=(kt == 0),
                                stop=False,
                            )
                        # bias: rank-1  ones (x) bv
                        nc.tensor.matmul(
                            ps[:], ones_row[:], bv_row[:], start=False, stop=True
                        )
                        nc.scalar.copy(v_sb[:, c * (TB // P) + s], ps[:])

            # ============ phase 2: attention + o-projection ============
            with ExitStack() as p2:
                wo_pool = p2.enter_context(tc.tile_pool(name="wop", bufs=1))
                apool = p2.enter_context(tc.tile_pool(name="apool", bufs=2))
                ptpool = p2.enter_context(tc.tile_pool(name="ptpool", bufs=4))
                spool = p2.enter_context(tc.tile_pool(name="spool", bufs=3))
                ostg = p2.enter_context(tc.tile_pool(name="ostg", bufs=3))
                psS = p2.enter_context(tc.tile_pool(name="psS", bufs=3, space="PSUM"))
                psPO = p2.enter_context(tc.tile_pool(name="psPO", bufs=2, space="PSUM"))
                psR = p2.enter_context(tc.tile_pool(name="psR", bufs=1, space="PSUM"))
                psC = p2.enter_context(tc.tile_pool(name="psC", bufs=2, space="PSUM"))

                wo_res = wo_pool.tile([P, H, D], fp16, name="wo_res")
                wo_loaded = [False]

                pts = {}
                po_h = {}
                S_h = {}
                att_tb = {}

                def col_off(tb, jt):
                    # q-columns left of the diagonal subtile are fully
                    # masked; skip them (fp16 moving keeps full rate at any
                    # width)
                    jl = jt - 4 * tb
                    if 1 <= jl <= 3:
                        return jl * P
                    return 0

                def emit_score(tb, h, jt):
                    off = col_off(tb, jt)
                    w = TB - off
                    s = psS.tile([P, TB], f32, tag="s")
                    nc.tensor.matmul(
                        s[:, :w],
                        kT_sb[:, h, jt * P : (jt + 1) * P],
                        qT_sb[:, h, tb * TB + off : (tb + 1) * TB],
                        start=True,
                        stop=True,
                    )
                    if jt >= 4 * tb:
                        # diagonal-band tile: after the off-trim, the first
                        # 128 columns are exactly this tile's diagonal block
                        nc.vector.tensor_tensor(
                            s[:, :P], s[:, :P], tri[:], mybir.AluOpType.add
                        )
                    pt = ptpool.tile([P, TB], fp16, tag="pt")
                    nc.scalar.activation(
                        pt[:, :w], s[:, :w], mybir.ActivationFunctionType.Exp
                    )
                    pts[(tb, h, jt)] = pt

                def emit_rp(tb, h, jt):
                    # softmax denominator: accumulate exp sums on the
                    # otherwise-idle DVE into S, then ONE ones-matmul per
                    # head (tail_a) partition-reduces it — instead of one
                    # ones-matmul per j-tile on the critical PE.
                    njt = 4 * (tb + 1)
                    off = col_off(tb, jt)
                    w = TB - off
                    pt = pts.pop((tb, h, jt))
                    if jt == 0:
                        po_h[(tb, h)] = psPO.tile(
                            [P, TB], f32, tag="po", name=f"po{tb}_{h}"
                        )
                        S_h[(tb, h)] = spool.tile(
                            [P, TB], fp16, tag="S", name=f"S{tb}_{h}"
                        )
                        nc.vector.tensor_copy(S_h[(tb, h)][:], pt[:])
                    else:
                        nc.vector.tensor_tensor(
                            S_h[(tb, h)][:, off:],
                            S_h[(tb, h)][:, off:],
                            pt[:, :w],
                            mybir.AluOpType.add,
                        )
                    nc.tensor.matmul(
                        po_h[(tb, h)][:, off:],
                        v_sb[:, jt, h * HD : (h + 1) * HD],
                        pt[:, :w],
                        start=(jt == 0),
                        stop=(jt == njt - 1),
                    )

                recips = {}

                def emit_tail_a(tb, h):
                    # partition-reduce the DVE-accumulated S and start the
                    # reciprocal; the broadcast runs two tasks later
                    # (emit_tail_b) so nothing in the PE queue waits on the
                    # DVE reciprocal
                    S = S_h.pop((tb, h))
                    rs = psR.tile([1, TB], f32, tag="rsum")
                    nc.tensor.matmul(rs[:], ones_col[:], S[:], start=True, stop=True)
                    recip = spool.tile([1, TB], f32r, tag="recip")
                    nc.vector.reciprocal(recip[:], rs[:])
                    recips[(tb, h)] = recip

                def emit_tail_b(tb, h):
                    po = po_h.pop((tb, h))
                    recip = recips.pop((tb, h))
                    bc = spool.tile([P, TB], f32r, tag="bc")
                    nc.gpsimd.partition_broadcast(bc[:], recip[:], channels=P)
                    nc.vector.tensor_tensor(
                        att_tb[tb][:, h, :], po[:], bc[:], mybir.AluOpType.mult
                    )

                oproj_queue = []

                def emit_oproj_group(tb, att_sb, ob, tt):
                    ps = psC.tile([P, TB], f32, tag="psC")
                    for h in range(H):
                        nc.tensor.matmul(
                            ps[:],
                            att_sb[:, h, tt * P : (tt + 1) * P],
                            wo_res[:, h, ob * TB : (ob + 1) * TB],
                            start=(h == 0),
                            stop=(h == H - 1),
                        )
                    ot = ostg.tile([P, TB], fp16, tag="ostg")
                    # stage on the DVE: GPSIMD cannot access PSUM, and ACT
                    # is the phase-2 critical engine (exp stream); fp16 S
                    # accumulation freed the DVE headroom this needs
                    nc.vector.tensor_copy(ot[:], ps[:])
                    # the LAST block's writes go out on the GPSIMD SWDGE
                    # queue so the SP queue retires early and the next
                    # loop iteration's x stream prefetches during the tail
                    eng = nc.gpsimd if tb == NTB - 1 else nc.sync
                    eng.dma_start(
                        o[
                            tb * TB + tt * P : tb * TB + (tt + 1) * P,
                            ob * TB : (ob + 1) * TB,
                        ],
                        ot[:],
                    )

                def emit_v_deferred(s):
                    # last chunk's v quarter, dropped into the ACT-paced
                    # tb0 stretch; copy-out on DVE (ACT paces tb0)
                    ps = psC.tile([P, F], f32, tag="psC")
                    for kt in range(KT):
                        nc.tensor.matmul(
                            ps[:],
                            last_xt[:, kt, s * P : (s + 1) * P],
                            wv_res[:, kt],
                            start=(kt == 0),
                            stop=False,
                        )
                    nc.tensor.matmul(
                        ps[:], ones_row[:], bv_row[:], start=False, stop=True
                    )
                    nc.vector.tensor_copy(
                        v_sb[:, (NTB - 1) * (TB // P) + s], ps[:]
                    )

                drain_rate = [0.0]

                def emit_oproj(tb):
                    # queue the 16 groups, spread EVENLY over the next
                    # block's tasks: the exp stream paces ACT at ~612ns/task
                    # vs ~426ns of score+PV on PE, so each task needs a
                    # fraction of an o-proj group to keep the PE the
                    # critical engine throughout
                    att_sb = att_tb.pop(tb)
                    for ob in range(D // TB):
                        for tt in range(TB // P):
                            oproj_queue.append((tb, att_sb, ob, tt))
                    ntasks_next = 4 * H * (tb + 2)
                    drain_rate[0] = 16.0 / ntasks_next

                # one flat software pipeline across (tb, head, j-tile): the
                # score matmul leads the PV step by two so the PE never
                # waits on ACT's exp; head tails (rsum/recip/bc/normalize)
                # are deferred two tasks past the head's last PV so their
                # DVE/ACT dependencies are ready when the PE reaches them.
                tasks = []
                for tb in range(NTB):
                    for h in range(H):
                        for jt in range(4 * (tb + 1)):
                            tasks.append((tb, h, jt))

                for tb in range(NTB):
                    att_tb[tb] = apool.tile(
                        [P, H, TB], fp16, tag="att", name=f"att{tb}"
                    )

                pending_a = []  # (due_i, tb, h)
                pending_b = []  # (due_i, tb, h)
                drain_credit = [0.0]

                def drain_tails(i):
                    while pending_a and pending_a[0][0] <= i:
                        _, ttb, th = pending_a.pop(0)
                        emit_tail_a(ttb, th)
                        pending_b.append((i + 2, ttb, th))
                    while pending_b and pending_b[0][0] <= i:
                        _, ttb, th = pending_b.pop(0)
                        emit_tail_b(ttb, th)
                        if th == H - 1:
                            emit_oproj(ttb)

                emit_score(*tasks[0])
                emit_score(*tasks[1])
                for i in range(2, len(tasks)):
                    emit_score(*tasks[i])
                    j = i - 2
                    tb, h, jt = tasks[j]
                    emit_rp(tb, h, jt)
                    if jt == 4 * (tb + 1) - 1:
                        pending_a.append((i + 2, tb, h))
                    if i == 6 and not wo_loaded[0]:
                        # wo on the SP queue: the ACT engine is the phase-2
                        # critical engine (exp stream) and must not issue it
                        for hh in range(H):
                            nc.sync.dma_start(wo_res[:, hh], wo[:, hh])
                        wo_loaded[0] = True
                    drain_tails(i)
                    if i in (3, 7, 11, 15):
                        emit_v_deferred((i - 3) // 4)
                    if oproj_queue:
                        drain_credit[0] += drain_rate[0]
                        while drain_credit[0] >= 1.0 and oproj_queue:
                            emit_oproj_group(*oproj_queue.pop(0))
                            drain_credit[0] -= 1.0
                emit_rp(*tasks[-2])
                emit_rp(*tasks[-1])
                # tasks[-1] ends the final head; queue its tail explicitly
                pending_a.append((0, NTB - 1, H - 1))
                for _, ttb, th in pending_a:
                    emit_tail_a(ttb, th)
                    pending_b.append((0, ttb, th))
                pending_a = []
                for _, ttb, th in pending_b:
                    emit_tail_b(ttb, th)
                    if th == H - 1:
                        emit_oproj(ttb)
                pending_b = []
                while oproj_queue:
                    emit_oproj_group(*oproj_queue.pop(0))

            p1sb.close()
            rep_ctx.close()

    nc.compile()
    _CACHE[key] = nc
    return nc


def _in_maps(hidden_states, Wq, bq, Wk, bk, Wv, bv, Wo, bo):
    nfp16 = np.float16
    hs = np.asarray(hidden_states, np.float32)
    Wq = np.asarray(Wq, np.float32)
    Wk = np.asarray(Wk, np.float32)
    Wv = np.asarray(Wv, np.float32)
    Wo = np.asarray(Wo, np.float32)
    bq = np.asarray(bq, np.float32)
    bk = np.asarray(bk, np.float32)
    bv = np.asarray(bv, np.float32)

    maps = []
    for b in range(B):
        xT = np.ascontiguousarray(hs[b].T).astype(nfp16)
        for g in range(G):
            sl = slice(g * F, (g + 1) * F)
            wqT = (Wq[sl, :].T * SCALE).astype(np.float32)   # (D, F)
            wkT = Wk[sl, :].T                                 # (D, F)
            wvT = Wv[sl, :].T                                 # (D, F)
            woT = Wo[:, sl].T                                 # (F, D)
            maps.append(
                {
                    "xT": xT,
                    "wq": np.ascontiguousarray(
                        wqT.reshape(KT, P, H, HD).transpose(1, 2, 0, 3)
                    ).astype(nfp16),
                    "wk": np.ascontiguousarray(
                        wkT.reshape(KT, P, H, HD).transpose(1, 2, 0, 3)
                    ).astype(nfp16),
                    "wv": np.ascontiguousarray(
                        wvT.reshape(KT, P, F).transpose(1, 0, 2)
                    ).astype(nfp16),
                    "wo": np.ascontiguousarray(
                        woT.reshape(H, HD, D).transpose(1, 0, 2)
                    ).astype(nfp16),
                    "bqv": np.ascontiguousarray(bq[sl] * SCALE),
                    "bkv": np.ascontiguousarray(bk[sl]),
                    "bvv": np.ascontiguousarray(bv[sl]).astype(nfp16),
                }
            )
    return maps


def kernel(hidden_states, Wq, bq, Wk, bk, Wv, bv, Wo, bo, **run_kwargs):
    nc = _build()
    maps = _in_maps(hidden_states, Wq, bq, Wk, bk, Wv, bv, Wo, bo)
    res = bass_utils.run_bass_kernel_spmd(
        nc, maps, core_ids=list(range(8)), **run_kwargs
    )
    bo = np.asarray(bo, np.float32)
    out = np.empty((B, L, D), np.float32)
    for b in range(B):
        acc = res.results[b * G]["o"].astype(np.float32)
        for g in range(1, G):
            acc = acc + res.results[b * G + g]["o"].astype(np.float32)
        out[b] = acc + bo[None, :]
    _CACHE["last_res"] = res
    return out


# revision 38
# speedup vs baseline: 1.1069x; 1.1069x over previous
"""Causal multi-head attention (B=2, L=2048, D=2048, NH=16, HD=128) on 8
Trainium2 NeuronCores.

Sharding: core c = b*4 + g handles batch b and head-group g (4 heads).
Each core computes q/k/v projections for its 512 features, causal
attention for its 4 heads, and the partial o-projection
attn_out @ Wo[:, g_cols].T -> [L, D] (fp16).  The host sums the 4
per-batch partials and adds bo.

v2 structure:
  phase 1: q,k AND v projections for the whole sequence from the same
           streamed x chunks (x read from HBM once); wq/wk/wv resident.
           qT/kT/v stored fp16.
  phase 2: flash-style causal attention in [k, q] layout fused with the
           partial o-projection.  Softmax row-sums are accumulated on
           the (otherwise idle) DVE engine across j-tiles, then reduced
           across partitions with ONE ones-matmul per (q-block, head)
           instead of one per j-tile.  pt/att/wo are fp16 so every
           matmul's moving operand runs at 1 cycle/row even when only
           128 columns survive the causal trim.

Matmul dtypes: moving operands are fp16 or f32r-with-moving>=256 (both
full rate).  PSUM accumulation is fp32 throughout.
"""

import sys

for _p in ("/opt/trn_rl_repo",):
    if _p not in sys.path:
        sys.path.insert(0, _p)

import numpy as np
from contextlib import ExitStack

import concourse.bass as bass  # noqa: F401
import concourse.tile as tile
from concourse import bacc, mybir
from concourse import bass_utils

P = 128
B, L, D = 2, 2048, 2048
NH, HD = 16, 128
SCALE = HD ** -0.5
G = 8 // B            # head-groups per batch = 4
H = 4                 # heads per core
F = H * HD            # 512 features per core
TB = 512              # token block (q-block)
NTB = L // TB         # 4
KT = D // P           # 16 contraction tiles for projections

f32r = mybir.dt.float32r
f32 = mybir.dt.float32
fp16 = mybir.dt.float16

_CACHE = {}


def _build(reps=1):
    key = ("nc", reps)
    if key in _CACHE:
        return _CACHE[key]

    nc = bacc.Bacc("TRN2", target_bir_lowering=False, debug=False, num_devices=8)

    xT = nc.dram_tensor("xT", [D, L], fp16, kind="ExternalInput").ap()
    wq = nc.dram_tensor("wq", [P, H, KT, HD], fp16, kind="ExternalInput").ap()
    wk = nc.dram_tensor("wk", [P, H, KT, HD], fp16, kind="ExternalInput").ap()
    wv = nc.dram_tensor("wv", [P, KT, F], fp16, kind="ExternalInput").ap()
    wo = nc.dram_tensor("wo", [P, H, D], fp16, kind="ExternalInput").ap()
    bqv = nc.dram_tensor("bqv", [F], f32r, kind="ExternalInput").ap()
    bkv = nc.dram_tensor("bkv", [F], f32r, kind="ExternalInput").ap()
    bvv = nc.dram_tensor("bvv", [F], fp16, kind="ExternalInput").ap()
    o = nc.dram_tensor("o", [L, D], fp16, kind="ExternalOutput").ap()

    xT3 = xT.rearrange("(kt p) t -> p kt t", p=P)

    with tile.TileContext(nc) as tc:
        with ExitStack() as ctx:
            ctx.enter_context(nc.allow_low_precision(reason="fp16 attention"))
            consts = ctx.enter_context(tc.tile_pool(name="consts", bufs=1))
            resid = ctx.enter_context(tc.tile_pool(name="resid", bufs=1))

            # ---- constants ----
            # triangular additive mask [P, P] in [k, q] orientation:
            # keep (0.0) where k_local <= q_local, else -1e30
            tri = consts.tile([P, P], f32, name="tri")
            nc.gpsimd.memset(tri[:], 0.0)
            nc.gpsimd.affine_select(
                out=tri[:],
                in_=tri[:],
                compare_op=mybir.AluOpType.is_ge,
                fill=-1e30,
                base=0,
                pattern=[[1, P]],
                channel_multiplier=-1,
            )

            # consts come from memset / the GPSIMD SWDGE queue: the SP and
            # ACT HWDGE queues are needed for the startup x/weight streams.
            # NeuronCC requires matmul operand dtypes to be both-32-bit or
            # both-non-32-bit: everything 16-bit is fp16 here.
            ones_col = consts.tile([P, 1], fp16)
            nc.gpsimd.memset(ones_col[:], 1.0)
            ones_row = consts.tile([1, P], fp16)
            nc.gpsimd.memset(ones_row[:], 1.0)
            bv_row = consts.tile([1, F], fp16)
            nc.gpsimd.dma_start(bv_row[:], bvv[None, :])
            # per-partition bias tiles for the q/k copies: [P, H]
            bq_pp = consts.tile([P, H], f32, name="bq_pp")
            nc.gpsimd.dma_start(bq_pp[:], bqv.rearrange("(h p) -> p h", p=P).bitcast(f32))
            bk_pp = consts.tile([P, H], f32, name="bk_pp")
            nc.gpsimd.dma_start(bk_pp[:], bkv.rearrange("(h p) -> p h", p=P).bitcast(f32))

            # ---- persistent activations (fp16) ----
            qT_sb = resid.tile([P, H, L], fp16, name="qT_sb")     # 2 MiB
            kT_sb = resid.tile([P, H, L], fp16, name="kT_sb")     # 2 MiB
            v_sb = resid.tile([P, L // P, F], fp16, name="v_sb")  # 2 MiB

            rep_ctx = ExitStack()
            if reps > 1:
                # timing mode: repeat the whole body in a hardware loop
                rep_ctx.enter_context(tc.For_i(0, reps, 1))

            # ============ phase 1: q, k, v projections ============
            # the SBUF pools stay open through phase 2: the last chunk's
            # v-quarters are deferred into the attention pipeline (they are
            # first needed by tb3) to fill the ACT-paced tb0 stretch
            p1sb = ExitStack()
            wres_pool = p1sb.enter_context(tc.tile_pool(name="wres", bufs=1))
            xpool = p1sb.enter_context(tc.tile_pool(name="xpool", bufs=3))
            with ExitStack() as p1:
                psA = p1.enter_context(tc.tile_pool(name="psA", bufs=4, space="PSUM"))
                psV = p1.enter_context(tc.tile_pool(name="psV", bufs=2, space="PSUM"))

                wq_res = wres_pool.tile([P, H, KT, HD], fp16, name="wq_res")
                wk_res = wres_pool.tile([P, H, KT, HD], fp16, name="wk_res")
                wv_res = wres_pool.tile([P, KT, F], fp16, name="wv_res")
                # split the weight stream across the two HWDGE queues in
                # consumption order: SP carries wq-h0 then the x pieces that
                # pace the first PSUM groups (plus the late heads); ACT
                # carries the early k/q heads and wv in parallel
                nc.sync.dma_start(wq_res[:, 0, :2], wq[:, 0, :2])

                for c in range(NTB):
                    lo = c * TB
                    xt = xpool.tile([P, KT, TB], fp16, tag="xT")
                    if c == 0:
                        # split the first chunk per-kt so the first PSUM
                        # group can start as soon as piece 0 lands; early
                        # weights stream on the ACT queue in parallel
                        nc.scalar.dma_start(wk_res[:, 0], wk[:, 0])
                        nc.scalar.dma_start(wq_res[:, 1], wq[:, 1])
                        nc.scalar.dma_start(wk_res[:, 1], wk[:, 1])
                        for kt in range(KT):
                            nc.sync.dma_start(xt[:, kt], xT3[:, kt, lo : lo + TB])
                            if kt == 1:
                                nc.sync.dma_start(wq_res[:, 0, 2:], wq[:, 0, 2:])
                        for vq in range(4):
                            nc.scalar.dma_start(
                                wv_res[:, 4 * vq : 4 * vq + 4],
                                wv[:, 4 * vq : 4 * vq + 4],
                            )
                        for hh in range(2, H):
                            nc.sync.dma_start(wq_res[:, hh], wq[:, hh])
                            nc.sync.dma_start(wk_res[:, hh], wk[:, hh])
                    else:
                        nc.sync.dma_start(xt[:], xT3[:, :, lo : lo + TB])
                    for h in range(H):
                        for wres, bias_pp, dst in (
                            (wq_res, bq_pp, qT_sb),
                            (wk_res, bk_pp, kT_sb),
                        ):
                            ps = psA.tile([P, TB], f32, tag="psA")
                            for kt in range(KT):
                                nc.tensor.matmul(
                                    ps[:],
                                    wres[:, h, kt],
                                    xt[:, kt],
                                    start=(kt == 0),
                                    stop=(kt == KT - 1),
                                )
                            nc.scalar.activation(
                                dst[:, h, lo : lo + TB],
                                ps[:],
                                mybir.ActivationFunctionType.Identity,
                                bias=bias_pp[:, h : h + 1],
                                scale=1.0,
                            )
                    if c == NTB - 1:
                        last_xt = xt
                        continue
                    for s in range(TB // P):
                        # v quarter: x slice stationary, wv moving
                        ps = psV.tile([P, F], f32, tag="psV")
                        for kt in range(KT):
                            nc.tensor.matmul(
                                ps[:],
                                xt[:, kt, s * P : (s + 1) * P],
                                wv_res[:, kt],
                                start=(kt == 0),
                                stop=False,
                            )
                        # bias: rank-1  ones (x) bv
                        nc.tensor.matmul(
                            ps[:], ones_row[:], bv_row[:], start=False, stop=True
                        )
                        nc.scalar.copy(v_sb[:, c * (TB // P) + s], ps[:])

            # ============ phase 2: attention + o-projection ============
            with ExitStack() as p2:
                wo_pool = p2.enter_context(tc.tile_pool(name="wop", bufs=1))
                apool = p2.enter_context(tc.tile_pool(name="apool", bufs=2))
                ptpool = p2.enter_context(tc.tile_pool(name="ptpool", bufs=4))
                spool = p2.enter_context(tc.tile_pool(name="spool", bufs=3))
                ostg = p2.enter_context(tc.tile_pool(name="ostg", bufs=3))
                psS = p2.enter_context(tc.tile_pool(name="psS", bufs=3, space="PSUM"))
                psPO = p2.enter_context(tc.tile_pool(name="psPO", bufs=2, space="PSUM"))
                psR = p2.enter_context(tc.tile_pool(name="psR", bufs=1, space="PSUM"))
                psC = p2.enter_context(tc.tile_pool(name="psC", bufs=2, space="PSUM"))

                wo_res = wo_pool.tile([P, H, D], fp16, name="wo_res")
                wo_loaded = [False]

                pts = {}
                po_h = {}
                S_h = {}
                att_tb = {}

                def col_off(tb, jt):
                    # q-columns left of the diagonal subtile are fully
                    # masked; skip them (fp16 moving keeps full rate at any
                    # width)
                    jl = jt - 4 * tb
                    if 1 <= jl <= 3:
                        return jl * P
                    return 0

                def emit_score(tb, h, jt):
                    off = col_off(tb, jt)
                    w = TB - off
                    s = psS.tile([P, TB], f32, tag="s")
                    nc.tensor.matmul(
                        s[:, :w],
                        kT_sb[:, h, jt * P : (jt + 1) * P],
                        qT_sb[:, h, tb * TB + off : (tb + 1) * TB],
                        start=True,
                        stop=True,
                    )
                    if jt >= 4 * tb:
                        # diagonal-band tile: after the off-trim, the first
                        # 128 columns are exactly this tile's diagonal block
                        nc.vector.tensor_tensor(
                            s[:, :P], s[:, :P], tri[:], mybir.AluOpType.add
                        )
                    pt = ptpool.tile([P, TB], fp16, tag="pt")
                    nc.scalar.activation(
                        pt[:, :w], s[:, :w], mybir.ActivationFunctionType.Exp
                    )
                    pts[(tb, h, jt)] = pt

                def emit_rp(tb, h, jt):
                    # softmax denominator: accumulate exp sums on the
                    # otherwise-idle DVE into S, then ONE ones-matmul per
                    # head (tail_a) partition-reduces it — instead of one
                    # ones-matmul per j-tile on the critical PE.
                    njt = 4 * (tb + 1)
                    off = col_off(tb, jt)
                    w = TB - off
                    pt = pts.pop((tb, h, jt))
                    if jt == 0:
                        po_h[(tb, h)] = psPO.tile(
                            [P, TB], f32, tag="po", name=f"po{tb}_{h}"
                        )
                        S_h[(tb, h)] = spool.tile(
                            [P, TB], fp16, tag="S", name=f"S{tb}_{h}"
                        )
                        nc.vector.tensor_copy(S_h[(tb, h)][:], pt[:])
                    else:
                        nc.vector.tensor_tensor(
                            S_h[(tb, h)][:, off:],
                            S_h[(tb, h)][:, off:],
                            pt[:, :w],
                            mybir.AluOpType.add,
                        )
                    nc.tensor.matmul(
                        po_h[(tb, h)][:, off:],
                        v_sb[:, jt, h * HD : (h + 1) * HD],
                        pt[:, :w],
                        start=(jt == 0),
                        stop=(jt == njt - 1),
                    )

                recips = {}

                def emit_tail_a(tb, h):
                    # partition-reduce the DVE-accumulated S and start the
                    # reciprocal; the broadcast runs two tasks later
                    # (emit_tail_b) so nothing in the PE queue waits on the
                    # DVE reciprocal
                    S = S_h.pop((tb, h))
                    rs = psR.tile([1, TB], f32, tag="rsum")
                    nc.tensor.matmul(rs[:], ones_col[:], S[:], start=True, stop=True)
                    recip = spool.tile([1, TB], f32r, tag="recip")
                    nc.vector.reciprocal(recip[:], rs[:])
                    recips[(tb, h)] = recip

                def emit_tail_b(tb, h):
                    po = po_h.pop((tb, h))
                    recip = recips.pop((tb, h))
                    bc = spool.tile([P, TB], f32r, tag="bc")
                    nc.gpsimd.partition_broadcast(bc[:], recip[:], channels=P)
                    nc.vector.tensor_tensor(
                        att_tb[tb][:, h, :], po[:], bc[:], mybir.AluOpType.mult
                    )

                oproj_queue = []

                def emit_oproj_group(tb, att_sb, ob, tt):
                    ps = psC.tile([P, TB], f32, tag="psC")
                    for h in range(H):
                        nc.tensor.matmul(
                            ps[:],
                            att_sb[:, h, tt * P : (tt + 1) * P],
                            wo_res[:, h, ob * TB : (ob + 1) * TB],
                            start=(h == 0),
                            stop=(h == H - 1),
                        )
                    ot = ostg.tile([P, TB], fp16, tag="ostg")
                    # stage on the DVE: GPSIMD cannot access PSUM, and ACT
                    # is the phase-2 critical engine (exp stream); fp16 S
                    # accumulation freed the DVE headroom this needs
                    nc.vector.tensor_copy(ot[:], ps[:])
                    # the LAST block's writes go out on the GPSIMD SWDGE
                    # queue so the SP queue retires early and the next
                    # loop iteration's x stream prefetches during the tail
                    eng = nc.gpsimd if tb == NTB - 1 else nc.sync
                    eng.dma_start(
                        o[
                            tb * TB + tt * P : tb * TB + (tt + 1) * P,
                            ob * TB : (ob + 1) * TB,
                        ],
                        ot[:],
                    )

                def emit_v_deferred(s):
                    # last chunk's v quarter, dropped into the ACT-paced
                    # tb0 stretch; copy-out on DVE (ACT paces tb0)
                    ps = psC.tile([P, F], f32, tag="psC")
                    for kt in range(KT):
                        nc.tensor.matmul(
                            ps[:],
                            last_xt[:, kt, s * P : (s + 1) * P],
                            wv_res[:, kt],
                            start=(kt == 0),
                            stop=False,
                        )
                    nc.tensor.matmul(
                        ps[:], ones_row[:], bv_row[:], start=False, stop=True
                    )
                    nc.vector.tensor_copy(
                        v_sb[:, (NTB - 1) * (TB // P) + s], ps[:]
                    )

                drain_rate = [0.0]

                def emit_oproj(tb):
                    # queue the 16 groups, spread EVENLY over the next
                    # block's tasks: the exp stream paces ACT at ~612ns/task
                    # vs ~426ns of score+PV on PE, so each task needs a
                    # fraction of an o-proj group to keep the PE the
                    # critical engine throughout
                    att_sb = att_tb.pop(tb)
                    for ob in range(D // TB):
                        for tt in range(TB // P):
                            oproj_queue.append((tb, att_sb, ob, tt))
                    ntasks_next = 4 * H * (tb + 2)
                    drain_rate[0] = 16.0 / ntasks_next

                # one flat software pipeline across (tb, head, j-tile): the
                # score matmul leads the PV step by two so the PE never
                # waits on ACT's exp; head tails (rsum/recip/bc/normalize)
                # are deferred two tasks past the head's last PV so their
                # DVE/ACT dependencies are ready when the PE reaches them.
                tasks = []
                for tb in range(NTB):
                    for h in range(H):
                        for jt in range(4 * (tb + 1)):
                            tasks.append((tb, h, jt))

                for tb in range(NTB):
                    att_tb[tb] = apool.tile(
                        [P, H, TB], fp16, tag="att", name=f"att{tb}"
                    )

                pending_a = []  # (due_i, tb, h)
                pending_b = []  # (due_i, tb, h)
                drain_credit = [0.0]

                def drain_tails(i):
                    while pending_a and pending_a[0][0] <= i:
                        _, ttb, th = pending_a.pop(0)
                        emit_tail_a(ttb, th)
                        pending_b.append((i + 2, ttb, th))
                    while pending_b and pending_b[0][0] <= i:
                        _, ttb, th = pending_b.pop(0)
                        emit_tail_b(ttb, th)
                        if th == H - 1:
                            emit_oproj(ttb)

                emit_score(*tasks[0])
                emit_score(*tasks[1])
                for i in range(2, len(tasks)):
                    emit_score(*tasks[i])
                    j = i - 2
                    tb, h, jt = tasks[j]
                    emit_rp(tb, h, jt)
                    if jt == 4 * (tb + 1) - 1:
                        pending_a.append((i + 2, tb, h))
                    if i == 6 and not wo_loaded[0]:
                        # wo on the SP queue: the ACT engine is the phase-2
                        # critical engine (exp stream) and must not issue it
                        for hh in range(H):
                            nc.sync.dma_start(wo_res[:, hh], wo[:, hh])
                        wo_loaded[0] = True
                    drain_tails(i)
                    if i in (3, 7, 11, 15):
                        emit_v_deferred((i - 3) // 4)
                    if oproj_queue:
                        drain_credit[0] += drain_rate[0]
                        while drain_credit[0] >= 1.0 and oproj_queue:
                            emit_oproj_group(*oproj_queue.pop(0))
                            drain_credit[0] -= 1.0
                emit_rp(*tasks[-2])
                emit_rp(*tasks[-1])
                # tasks[-1] ends the final head; queue its tail explicitly
                pending_a.append((0, NTB - 1, H - 1))
                for _, ttb, th in pending_a:
                    emit_tail_a(ttb, th)
                    pending_b.append((0, ttb, th))
                pending_a = []
                for _, ttb, th in pending_b:
                    emit_tail_b(ttb, th)
                    if th == H - 1:
                        emit_oproj(ttb)
                pending_b = []
                while oproj_queue:
                    emit_oproj_group(*oproj_queue.pop(0))

            p1sb.close()
            rep_ctx.close()

    nc.compile()
    _CACHE[key] = nc
    return nc


def _in_maps(hidden_states, Wq, bq, Wk, bk, Wv, bv, Wo, bo):
    nfp16 = np.float16
    hs = np.asarray(hidden_states, np.float32)
    Wq = np.asarray(Wq, np.float32)
    Wk = np.asarray(Wk, np.float32)
    Wv = np.asarray(Wv, np.float32)
    Wo = np.asarray(Wo, np.float32)
    bq = np.asarray(bq, np.float32)
    bk = np.asarray(bk, np.float32)
    bv = np.asarray(bv, np.float32)

    maps = []
    for b in range(B):
        xT = np.ascontiguousarray(hs[b].T).astype(nfp16)
        for g in range(G):
            sl = slice(g * F, (g + 1) * F)
            wqT = (Wq[sl, :].T * SCALE).astype(np.float32)   # (D, F)
            wkT = Wk[sl, :].T                                 # (D, F)
            wvT = Wv[sl, :].T                                 # (D, F)
            woT = Wo[:, sl].T                                 # (F, D)
            maps.append(
                {
                    "xT": xT,
                    "wq": np.ascontiguousarray(
                        wqT.reshape(KT, P, H, HD).transpose(1, 2, 0, 3)
                    ).astype(nfp16),
                    "wk": np.ascontiguousarray(
                        wkT.reshape(KT, P, H, HD).transpose(1, 2, 0, 3)
                    ).astype(nfp16),
                    "wv": np.ascontiguousarray(
                        wvT.reshape(KT, P, F).transpose(1, 0, 2)
                    ).astype(nfp16),
                    "wo": np.ascontiguousarray(
                        woT.reshape(H, HD, D).transpose(1, 0, 2)
                    ).astype(nfp16),
                    "bqv": np.ascontiguousarray(bq[sl] * SCALE),
                    "bkv": np.ascontiguousarray(bk[sl]),
                    "bvv": np.ascontiguousarray(bv[sl]).astype(nfp16),
                }
            )
    return maps


def kernel(hidden_states, Wq, bq, Wk, bk, Wv, bv, Wo, bo, **run_kwargs):
    nc = _build()
    maps = _in_maps(hidden_states, Wq, bq, Wk, bk, Wv, bv, Wo, bo)
    res = bass_utils.run_bass_kernel_spmd(
        nc, maps, core_ids=list(range(8)), **run_kwargs
    )
    bo = np.asarray(bo, np.float32)
    out = np.empty((B, L, D), np.float32)
    for b in range(B):
        acc = res.results[b * G]["o"].astype(np.float32)
        for g in range(1, G):
            acc = acc + res.results[b * G + g]["o"].astype(np.float32)
        out[b] = acc + bo[None, :]
    _CACHE["last_res"] = res
    return out
